# revision 1
# baseline (speedup 1.0000x reference)
"""AttentionPooling (segment mean -> att = <x_i, coarse[batch_i]> -> weighted
segment mean -> Linear) on 8 Trainium2 NeuronCores.

Strategy
--------
`batch` is sorted and host-visible inside kernel(), so ALL index structure is
resolved on the host:

* The 8192 segments are bin-packed into 512 groups of exactly 16 segments,
  each group padded to 4096 rows (32 sub-tiles of 128 rows).  Rows are
  permuted so every group is contiguous; pad rows are zero.  Each core owns
  64 groups -> perfectly uniform SPMD program, no collectives (a segment
  never straddles cores).
* Per group the device does, on SBUF-resident tiles loaded once:
    pass A:  sums[slot, d]  += P_st^T @ x_st          (P = one-hot, stationary)
             C_g = sums * (1/count)  -> transpose -> CT_g [d, 16]
    pass B:  A_st[rows,16] = x_st @ CT_g              (x^T stationary)
             Q = A * P    (one fused DVE multiply per 4 sub-tiles)
             pooled[slot, d] += Q_st^T @ x_st
             pooledT[d, slot] = transpose(pooled * 1/count)
    final :  out[seg, :] = pooledT^T @ W^T + b        (8 window matmuls)
* x is fed in bf16 in both layouts (rows-major and transposed); all
  reductions accumulate in fp32 PSUM; 1/count scaling is fp32.
"""

import os

import numpy as np
import ml_dtypes

import concourse.bass as bass
import concourse.mybir as mybir
import concourse.tile as tile
from concourse.bass_utils import run_bass_kernel_spmd
from concourse.vector_clock import ScopedClock

BF16 = mybir.dt.bfloat16
F32 = mybir.dt.float32

N_CORES = 8
B_SEGS = 8192
D = 128
G = 16                  # segments (slots) per group
ST = 32                 # 128-row sub-tiles per group
GROUP_ROWS = ST * 128   # 4096
N_GROUPS = B_SEGS // G  # 512
GROUPS_PER_CORE = N_GROUPS // N_CORES  # 64
CORE_ROWS = GROUPS_PER_CORE * GROUP_ROWS  # 262144

LAST_RESULT = None  # BassKernelResults of the most recent run (for test.py)

_PATCHED = False


def _patch_tile_tail():
    """The walrus build in this container only lowers ONE sync-wait per
    instruction.  Tile routinely emits multi-wait instructions, so (a) split
    every scheduled instruction's extra waits onto injected same-engine NOPs
    (engines execute their stream in order, so a wait on a preceding NOP is
    equivalent), and (b) do the same for the TileContext exit drain."""
    global _PATCHED
    if _PATCHED:
        return
    _PATCHED = True

    orig_lower = tile.TileContext._lower_ordered_insts

    def _lower_ordered_insts(self, ordered):
        nid = [0]
        for bb_name, insts in ordered.items():
            new = []
            for inst in insts:
                si = inst.sync_info
                if si is not None and si.on_wait and len(si.on_wait) > 1:
                    waits = list(si.on_wait)
                    for w in waits[:-1]:
                        nid[0] += 1
                        nop = mybir.InstNoOp(
                            name=f"splitw-{nid[0]}-{inst.name}",
                            engine=inst.engine,
                            sync_info=mybir.SyncInfo(on_wait=[w], on_update=[]),
                            bass_nofuse=True,
                        )
                        new.append(nop)
                    si.on_wait = [waits[-1]]
                new.append(inst)
            ordered[bb_name] = new
        return orig_lower(self, ordered)

    tile.TileContext._lower_ordered_insts = _lower_ordered_insts

    def _drain_and_barrier(self, tick_clock, wait_clock):
        nc = self.nc
        probe = nc.sync.nop(nofuse=True, hint="tail_wait0")
        wait_clock.add_sem_waits(
            probe.ins, ScopedClock({None: tick_clock.global_clock})
        )
        si = probe.ins.sync_info
        waits = list(si.on_wait or []) if si is not None else []
        if len(waits) > 1:
            si.on_wait = waits[:1]
            for w in waits[1:]:
                n2 = nc.sync.nop(nofuse=True, hint="tail_wait")
                n2.ins.sync_info = mybir.SyncInfo(on_wait=[w], on_update=[])
        nc.sync.drain()
        nc.all_engine_barrier()
        popped = nc._tile_sem_poison_stack.pop()
        assert popped is self._sem_poison
        nc.clear_and_free_semaphores(list(self.sems.allocated().values()))
        nc.all_engine_barrier()

    tile.TileContext._drain_and_barrier = _drain_and_barrier


# --------------------------------------------------------------------------
# host-side packing
# --------------------------------------------------------------------------

def _pack_segments(counts):
    """Assign each segment to a (group, slot).  512 groups x 16 slots, rows
    per group <= GROUP_ROWS.  Balanced LPT dealing: 16 rounds; each round
    hands the next 512 largest segments to the currently lightest groups."""
    order = np.argsort(-counts, kind="stable")
    loads = np.zeros(N_GROUPS, dtype=np.int64)
    seg_ids = np.empty((N_GROUPS, G), dtype=np.int64)
    for r in range(G):
        chunk = order[r * N_GROUPS:(r + 1) * N_GROUPS]
        grp_order = np.argsort(loads, kind="stable")
        seg_ids[grp_order, r] = chunk
        loads[grp_order] += counts[chunk]
    assert loads.max() <= GROUP_ROWS, (
        f"group overflow: {loads.max()} > {GROUP_ROWS}"
    )
    return seg_ids  # [512, 16] segment id per (group, slot)


def _host_prepare(x, batch, W, b):
    counts = np.bincount(batch, minlength=B_SEGS).astype(np.int64)
    seg_start = np.concatenate([[0], np.cumsum(counts)[:-1]])
    seg_ids = _pack_segments(counts)                       # [512, 16]

    flat_segs = seg_ids.reshape(-1)                        # packed order
    flat_counts = counts[flat_segs]
    # destination start of each packed segment
    within = flat_counts.reshape(N_GROUPS, G)
    offs = np.cumsum(within, axis=1) - within              # [512, 16]
    dest_start = (np.arange(N_GROUPS)[:, None] * GROUP_ROWS + offs).reshape(-1)
    src_start = seg_start[flat_segs]

    total = int(flat_counts.sum())
    assert total == x.shape[0]
    rag = np.arange(total, dtype=np.int64) - np.repeat(
        np.cumsum(flat_counts) - flat_counts, flat_counts
    )
    valid_dest = np.repeat(dest_start, flat_counts) + rag
    valid_src = np.repeat(src_start, flat_counts) + rag

    n_pad = N_GROUPS * GROUP_ROWS
    x_bf = x.astype(ml_dtypes.bfloat16)
    x_pad = np.zeros((n_pad, D), dtype=ml_dtypes.bfloat16)
    x_pad[valid_dest] = x_bf[valid_src]
    xt_pad = np.ascontiguousarray(x_pad.T)                 # [128, n_pad]

    slotvec = np.zeros(n_pad, dtype=np.float32)
    slot_of_seg = np.repeat(
        np.tile(np.arange(G, dtype=np.float32), N_GROUPS), flat_counts
    )
    slotvec[valid_dest] = slot_of_seg

    invc = (1.0 / np.maximum(counts, 1)).astype(np.float32)
    invc_packed = invc[flat_segs].reshape(N_GROUPS, G)     # [512, 16]

    iota = np.tile(
        np.tile(np.arange(G, dtype=np.float32), 4)[None, :], (128, 1)
    )                                                      # [128, 64]
    consts = {
        "iota": np.ascontiguousarray(iota),
        "idb": np.eye(G, dtype=ml_dtypes.bfloat16),
        "idf": np.eye(G, dtype=np.float32),
        "wt": np.ascontiguousarray(W.T.astype(np.float32)),
        "bb": np.ascontiguousarray(b.astype(np.float32).reshape(1, D)),
        "ones": np.ones((1, D), dtype=np.float32),
    }

    in_maps = []
    for c in range(N_CORES):
        r0, r1 = c * CORE_ROWS, (c + 1) * CORE_ROWS
        g0, g1 = c * GROUPS_PER_CORE, (c + 1) * GROUPS_PER_CORE
        m = {
            "xg": np.ascontiguousarray(x_pad[r0:r1]),
            "xt": np.ascontiguousarray(xt_pad[:, r0:r1]),
            # slot[p, st] = slotvec[st*128 + p] (core-local)
            "slot": np.ascontiguousarray(
                slotvec[r0:r1].reshape(-1, 128).T
            ),
            "invc": np.ascontiguousarray(invc_packed[g0:g1].T),  # [16, 64]
        }
        m.update(consts)
        in_maps.append(m)

    return in_maps, seg_ids


# --------------------------------------------------------------------------
# device program
# --------------------------------------------------------------------------

def _build_program(groups=GROUPS_PER_CORE):
    _patch_tile_tail()
    nc = bass.Bass("TRN2", debug=False)
    rows = groups * GROUP_ROWS

    xg_h = nc.dram_tensor("xg", [rows, D], BF16, kind="ExternalInput")
    xt_h = nc.dram_tensor("xt", [D, rows], BF16, kind="ExternalInput")
    slot_h = nc.dram_tensor("slot", [128, groups * ST], F32, kind="ExternalInput")
    invc_h = nc.dram_tensor("invc", [G, groups], F32, kind="ExternalInput")
    iota_h = nc.dram_tensor("iota", [128, 64], F32, kind="ExternalInput")
    idb_h = nc.dram_tensor("idb", [G, G], BF16, kind="ExternalInput")
    idf_h = nc.dram_tensor("idf", [G, G], F32, kind="ExternalInput")
    wt_h = nc.dram_tensor("wt", [D, D], F32, kind="ExternalInput")
    bb_h = nc.dram_tensor("bb", [1, D], F32, kind="ExternalInput")
    ones_h = nc.dram_tensor("ones", [1, D], F32, kind="ExternalInput")
    out_h = nc.dram_tensor("out", [groups * G, D], F32, kind="ExternalOutput")

    xg = xg_h.ap().rearrange("(gr st p) d -> gr p st d", p=128, st=ST)
    # xg[g] : [128, ST, D] ; row (g*4096 + st*128 + p) at [p, st, :]

    mult = mybir.AluOpType.mult
    is_eq = mybir.AluOpType.is_equal

    with tile.TileContext(nc) as tc:
        from contextlib import ExitStack
        with ExitStack() as ctx:
            cpool = ctx.enter_context(tc.tile_pool(name="consts", bufs=1))
            slot_t = cpool.tile([128, groups * ST], F32)
            nc.sync.dma_start(out=slot_t[:], in_=slot_h.ap()[:])
            invc_t = cpool.tile([G, groups], F32)
            nc.sync.dma_start(out=invc_t[:], in_=invc_h.ap()[:])
            iota_t = cpool.tile([128, 64], F32)
            nc.sync.dma_start(out=iota_t[:], in_=iota_h.ap()[:])
            idb_t = cpool.tile([G, G], BF16)
            nc.sync.dma_start(out=idb_t[:], in_=idb_h.ap()[:])
            idf_t = cpool.tile([G, G], F32)
            nc.sync.dma_start(out=idf_t[:], in_=idf_h.ap()[:])
            wt_t = cpool.tile([D, D], F32)
            nc.sync.dma_start(out=wt_t[:], in_=wt_h.ap()[:])
            bb_t = cpool.tile([1, D], F32)
            nc.sync.dma_start(out=bb_t[:], in_=bb_h.ap()[:])
            ones_t = cpool.tile([1, D], F32)
            nc.sync.dma_start(out=ones_t[:], in_=ones_h.ap()[:])

            pooledT = cpool.tile([128, groups * G], F32)  # persistent result

            xpool = ctx.enter_context(tc.tile_pool(name="x", bufs=3))
            xtpool = ctx.enter_context(tc.tile_pool(name="xt", bufs=3))
            ppool = ctx.enter_context(tc.tile_pool(name="p", bufs=20))
            qpool = ctx.enter_context(tc.tile_pool(name="q", bufs=4))
            ctpool = ctx.enter_context(tc.tile_pool(name="ct", bufs=groups))
            cgpool = ctx.enter_context(tc.tile_pool(name="cg", bufs=2))
            pgpool = ctx.enter_context(tc.tile_pool(name="pg", bufs=2))

            with ExitStack() as psctx:
                psa_pool = psctx.enter_context(
                    tc.tile_pool(name="psA", bufs=2, space="PSUM"))
                psq_pool = psctx.enter_context(
                    tc.tile_pool(name="psAq", bufs=2, space="PSUM"))
                psp_pool = psctx.enter_context(
                    tc.tile_pool(name="psPool", bufs=2, space="PSUM"))
                pst_pool = psctx.enter_context(
                    tc.tile_pool(name="psT", bufs=2, space="PSUM"))

                state = {}  # per-group live tiles handed from pass A to B

                def emit_passA(g):
                    xg_t = xpool.tile([128, ST * D], BF16)
                    nc.sync.dma_start(out=xg_t[:], in_=xg[g])
                    xt_t = xtpool.tile([128, GROUP_ROWS], BF16)
                    nc.sync.dma_start(
                        out=xt_t[:],
                        in_=xt_h.ap()[:, g * GROUP_ROWS:(g + 1) * GROUP_ROWS],
                    )
                    psA = psa_pool.tile([G, D], F32, space="PSUM")
                    pquads = []
                    for q in range(8):
                        pq = ppool.tile([128, 64], BF16, tag="p")
                        nc.vector.tensor_tensor(
                            out=pq[:],
                            in0=slot_t[:, g * ST + 4 * q: g * ST + 4 * q + 4]
                                .to_broadcast([128, 4, G]),
                            in1=iota_t[:],
                            op=is_eq,
                        )
                        pquads.append(pq)
                        for k in range(4):
                            st = 4 * q + k
                            nc.tensor.matmul(
                                out=psA[:],
                                lhsT=pq[:, k * G:(k + 1) * G],
                                rhs=xg_t[:, st * D:(st + 1) * D],
                                start=(st == 0),
                                stop=(st == ST - 1),
                            )
                    # C_g = sums * 1/count, then transpose to [d, 16]
                    cg = cgpool.tile([G, D], BF16)
                    nc.vector.tensor_scalar(
                        out=cg[:], in0=psA[:],
                        scalar1=invc_t[:, g:g + 1], scalar2=None, op0=mult,
                    )
                    psT = pst_pool.tile([128, G], BF16, space="PSUM")
                    nc.tensor.transpose(out=psT[:], in_=cg[:], identity=idb_t[:])
                    ct = ctpool.tile([128, G], BF16, tag="ct")
                    nc.scalar.copy(out=ct[:], in_=psT[:])
                    state[g] = (xg_t, xt_t, pquads, ct)

                def emit_passB(g):
                    xg_t, xt_t, pquads, ct = state.pop(g)
                    psPool = psp_pool.tile([G, D], F32, space="PSUM")
                    for q in range(8):
                        psAq = psq_pool.tile([128, 64], F32, space="PSUM")
                        for k in range(4):
                            st = 4 * q + k
                            nc.tensor.matmul(
                                out=psAq[:, k * G:(k + 1) * G],
                                lhsT=xt_t[:, st * 128:(st + 1) * 128],
                                rhs=ct[:],
                                start=True, stop=True,
                            )
                        qq = qpool.tile([128, 64], BF16, tag="q")
                        nc.vector.tensor_tensor(
                            out=qq[:], in0=psAq[:], in1=pquads[q][:], op=mult,
                        )
                        for k in range(4):
                            st = 4 * q + k
                            nc.tensor.matmul(
                                out=psPool[:],
                                lhsT=qq[:, k * G:(k + 1) * G],
                                rhs=xg_t[:, st * D:(st + 1) * D],
                                start=(st == 0),
                                stop=(st == ST - 1),
                            )
                    pg = pgpool.tile([G, D], F32)
                    nc.vector.tensor_scalar(
                        out=pg[:], in0=psPool[:],
                        scalar1=invc_t[:, g:g + 1], scalar2=None, op0=mult,
                    )
                    psT = pst_pool.tile([128, G], F32, space="PSUM")
                    nc.tensor.transpose(out=psT[:], in_=pg[:], identity=idf_t[:])
                    nc.scalar.copy(out=pooledT[:, g * G:(g + 1) * G], in_=psT[:])

                for g in range(groups):
                    emit_passA(g)
                    if g >= 1:
                        emit_passB(g - 1)
                emit_passB(groups - 1)

            # final linear: out[seg, :] = pooled @ W^T + b  (windows of 128)
            with (
                tc.tile_pool(name="psO", bufs=2, space="PSUM") as pso_pool,
                tc.tile_pool(name="ob", bufs=2) as ob_pool,
            ):
                n_segs = groups * G
                for w0 in range(0, n_segs, 128):
                    wn = min(128, n_segs - w0)
                    pso = pso_pool.tile([128, D], F32, space="PSUM")
                    nc.tensor.matmul(
                        out=pso[:wn, :], lhsT=pooledT[:, w0:w0 + wn],
                        rhs=wt_t[:], start=True, stop=False,
                    )
                    nc.tensor.matmul(
                        out=pso[:wn, :], lhsT=ones_t[:, :wn], rhs=bb_t[:],
                        start=False, stop=True,
                    )
                    ob = ob_pool.tile([128, D], F32)
                    nc.vector.tensor_copy(out=ob[:wn, :], in_=pso[:wn, :])
                    nc.sync.dma_start(
                        out=out_h.ap()[w0:w0 + wn, :], in_=ob[:wn, :],
                    )

    return nc


# --------------------------------------------------------------------------
# entry point
# --------------------------------------------------------------------------

def kernel(x, batch, W, b, num_segments):
    global LAST_RESULT
    x = np.asarray(x)
    batch = np.asarray(batch)
    W = np.asarray(W, dtype=np.float32)
    b = np.asarray(b, dtype=np.float32)

    in_maps, seg_ids = _host_prepare(x, batch, W, b)
    nc = _build_program()

    trace = bool(int(os.environ.get("KERNEL_TRACE", "0")))
    res = run_bass_kernel_spmd(
        nc, in_maps, core_ids=list(range(N_CORES)),
        trace=trace, trace_cores=[0] if trace else None,
    )
    LAST_RESULT = res

    out = np.empty((B_SEGS, D), dtype=np.float32)
    packed = np.concatenate([res.results[c]["out"] for c in range(N_CORES)], axis=0)
    out[seg_ids.reshape(-1)] = packed
    return out



# revision 2
# speedup vs baseline: 1.3306x; 1.3306x over previous
"""AttentionPooling (segment mean -> att = <x_i, coarse[batch_i]> -> weighted
segment mean -> Linear) on 8 Trainium2 NeuronCores.

Strategy
--------
`batch` is sorted and host-visible inside kernel(), so ALL index structure is
resolved on the host:

* The 8192 segments are bin-packed into 512 groups of exactly 16 segments,
  each group padded to 4096 rows (32 sub-tiles of 128 rows).  Rows are
  permuted so every group is contiguous; pad rows are zero.  Each core owns
  64 groups -> perfectly uniform SPMD program, no collectives (a segment
  never straddles cores).
* Row (p, st) of a group maps to packed row p*32+st, so the rows-layout DMA
  moves 8 KiB contiguous per partition (128 descriptors per tile instead of
  4096).  The transposed copy is column-permuted on the host to match.
* All matmuls keep x as the 128-column STATIONARY operand (FWL-eligible)
  and stream 16-column moving operands (issue-floor bound, ~25 ns/pair):
    pass A:  sumsT[d, slot]   += xg_st^T @ P_st     (P one-hot, moving)
             ct = sumsT * (1/count)                 [d, 16] bf16, no transpose
    pass B:  att[rows, slot]   = xt_st^T @ ct       (xt stationary)
             Q = att * P        (one DVE multiply per 8 sub-tiles)
             pooledT[d, slot] += xg_st^T @ Q_st
             pooledT slice * (1/count) -> SBUF      (DVE, no transpose)
    final :  out[seg, :] = pooledT^T @ W^T + b      (8 window matmuls)
* x is fed in bf16 in both layouts; all reductions accumulate in fp32 PSUM.
"""

import os

import numpy as np
import ml_dtypes

import concourse.bass as bass
import concourse.mybir as mybir
import concourse.tile as tile
from concourse.bass_utils import run_bass_kernel_spmd
from concourse.vector_clock import ScopedClock

BF16 = mybir.dt.bfloat16
F32 = mybir.dt.float32

N_CORES = 8
B_SEGS = 8192
D = 128
G = 16                  # segments (slots) per group
ST = 32                 # 128-row sub-tiles per group
GROUP_ROWS = ST * 128   # 4096
N_GROUPS = B_SEGS // G  # 512
GROUPS_PER_CORE = N_GROUPS // N_CORES  # 64
CORE_ROWS = GROUPS_PER_CORE * GROUP_ROWS  # 262144

LAST_RESULT = None  # BassKernelResults of the most recent run (for test.py)

_PATCHED = False


def _patch_tile_tail():
    """The walrus build in this container only lowers ONE sync-wait per
    instruction.  Tile routinely emits multi-wait instructions, so (a) split
    every scheduled instruction's extra waits onto injected same-engine NOPs
    (engines execute their stream in order, so a wait on a preceding NOP is
    equivalent), and (b) do the same for the TileContext exit drain."""
    global _PATCHED
    if _PATCHED:
        return
    _PATCHED = True

    orig_lower = tile.TileContext._lower_ordered_insts

    def _lower_ordered_insts(self, ordered):
        nid = [0]
        for bb_name, insts in ordered.items():
            new = []
            for inst in insts:
                si = inst.sync_info
                if si is not None and si.on_wait and len(si.on_wait) > 1:
                    waits = list(si.on_wait)
                    for w in waits[:-1]:
                        nid[0] += 1
                        nop = mybir.InstNoOp(
                            name=f"splitw-{nid[0]}-{inst.name}",
                            engine=inst.engine,
                            sync_info=mybir.SyncInfo(on_wait=[w], on_update=[]),
                            bass_nofuse=True,
                        )
                        new.append(nop)
                    si.on_wait = [waits[-1]]
                new.append(inst)
            ordered[bb_name] = new
        return orig_lower(self, ordered)

    tile.TileContext._lower_ordered_insts = _lower_ordered_insts

    def _drain_and_barrier(self, tick_clock, wait_clock):
        nc = self.nc
        probe = nc.sync.nop(nofuse=True, hint="tail_wait0")
        wait_clock.add_sem_waits(
            probe.ins, ScopedClock({None: tick_clock.global_clock})
        )
        si = probe.ins.sync_info
        waits = list(si.on_wait or []) if si is not None else []
        if len(waits) > 1:
            si.on_wait = waits[:1]
            for w in waits[1:]:
                n2 = nc.sync.nop(nofuse=True, hint="tail_wait")
                n2.ins.sync_info = mybir.SyncInfo(on_wait=[w], on_update=[])
        nc.sync.drain()
        nc.all_engine_barrier()
        popped = nc._tile_sem_poison_stack.pop()
        assert popped is self._sem_poison
        nc.clear_and_free_semaphores(list(self.sems.allocated().values()))
        nc.all_engine_barrier()

    tile.TileContext._drain_and_barrier = _drain_and_barrier


# --------------------------------------------------------------------------
# host-side packing
# --------------------------------------------------------------------------

def _pack_segments(counts):
    """Assign each segment to a (group, slot).  512 groups x 16 slots, rows
    per group <= GROUP_ROWS.  Balanced LPT dealing: 16 rounds; each round
    hands the next 512 largest segments to the currently lightest groups."""
    order = np.argsort(-counts, kind="stable")
    loads = np.zeros(N_GROUPS, dtype=np.int64)
    seg_ids = np.empty((N_GROUPS, G), dtype=np.int64)
    for r in range(G):
        chunk = order[r * N_GROUPS:(r + 1) * N_GROUPS]
        grp_order = np.argsort(loads, kind="stable")
        seg_ids[grp_order, r] = chunk
        loads[grp_order] += counts[chunk]
    assert loads.max() <= GROUP_ROWS, (
        f"group overflow: {loads.max()} > {GROUP_ROWS}"
    )
    return seg_ids  # [512, 16] segment id per (group, slot)


def _host_prepare(x, batch, W, b):
    counts = np.bincount(batch, minlength=B_SEGS).astype(np.int64)
    seg_start = np.concatenate([[0], np.cumsum(counts)[:-1]])
    seg_ids = _pack_segments(counts)                       # [512, 16]

    flat_segs = seg_ids.reshape(-1)                        # packed order
    flat_counts = counts[flat_segs]
    # destination start of each packed segment
    within = flat_counts.reshape(N_GROUPS, G)
    offs = np.cumsum(within, axis=1) - within              # [512, 16]
    dest_start = (np.arange(N_GROUPS)[:, None] * GROUP_ROWS + offs).reshape(-1)
    src_start = seg_start[flat_segs]

    total = int(flat_counts.sum())
    assert total == x.shape[0]
    rag = np.arange(total, dtype=np.int64) - np.repeat(
        np.cumsum(flat_counts) - flat_counts, flat_counts
    )
    valid_dest = np.repeat(dest_start, flat_counts) + rag
    valid_src = np.repeat(src_start, flat_counts) + rag

    n_pad = N_GROUPS * GROUP_ROWS
    x_bf = x.astype(ml_dtypes.bfloat16)
    x_pad = np.zeros((n_pad, D), dtype=ml_dtypes.bfloat16)
    x_pad[valid_dest] = x_bf[valid_src]
    # Device row (p, st) of group g = packed row g*4096 + p*32 + st:
    # the rows layout is x_pad itself (8 KiB contiguous per partition); the
    # transposed copy is column-permuted to st-major so sub-tile st's
    # columns are the 128 rows {p*32+st}.
    xt_pad = np.ascontiguousarray(
        x_pad.reshape(N_GROUPS, 128, ST, D)
        .transpose(0, 2, 1, 3)
        .reshape(n_pad, D)
        .T
    )                                                      # [128, n_pad]

    slotvec = np.zeros(n_pad, dtype=np.float32)
    slot_of_seg = np.repeat(
        np.tile(np.arange(G, dtype=np.float32), N_GROUPS), flat_counts
    )
    slotvec[valid_dest] = slot_of_seg
    # slot[p, g*ST+st] = slotvec[g*4096 + p*32 + st]
    slot_pmaj = (
        slotvec.reshape(N_GROUPS, 128, ST)
        .transpose(1, 0, 2)
        .reshape(128, N_GROUPS * ST)
    )

    invc = (1.0 / np.maximum(counts, 1)).astype(np.float32)
    invc_packed = invc[flat_segs].reshape(N_GROUPS, G)     # [512, 16]

    # iota8[p, 16*j + s] = s  (compare target for 8 sub-tiles' one-hots)
    iota8 = np.tile(
        np.tile(np.arange(G, dtype=np.float32), 8)[None, :], (128, 1)
    )                                                      # [128, 128]
    consts = {
        "iota8": np.ascontiguousarray(iota8),
        "wt": np.ascontiguousarray(W.T.astype(np.float32)),
        "bb": np.ascontiguousarray(b.astype(np.float32).reshape(1, D)),
        "ones": np.ones((1, D), dtype=np.float32),
    }

    in_maps = []
    for c in range(N_CORES):
        r0, r1 = c * CORE_ROWS, (c + 1) * CORE_ROWS
        g0, g1 = c * GROUPS_PER_CORE, (c + 1) * GROUPS_PER_CORE
        m = {
            "xg": np.ascontiguousarray(x_pad[r0:r1]),
            "xt": np.ascontiguousarray(xt_pad[:, r0:r1]),
            "slot": np.ascontiguousarray(
                slot_pmaj[:, g0 * ST:g1 * ST]
            ),
            # invcb[p, g_local*16 + s] = 1/count of (group, slot)
            "invcb": np.ascontiguousarray(
                np.tile(
                    invc_packed[g0:g1].reshape(1, -1), (128, 1)
                )
            ),
        }
        m.update(consts)
        in_maps.append(m)

    return in_maps, seg_ids


# --------------------------------------------------------------------------
# device program
# --------------------------------------------------------------------------

def _build_program(groups=GROUPS_PER_CORE):
    _patch_tile_tail()
    nc = bass.Bass("TRN2", debug=False)
    rows = groups * GROUP_ROWS

    xg_h = nc.dram_tensor("xg", [rows, D], BF16, kind="ExternalInput")
    xt_h = nc.dram_tensor("xt", [D, rows], BF16, kind="ExternalInput")
    slot_h = nc.dram_tensor("slot", [128, groups * ST], F32, kind="ExternalInput")
    invcb_h = nc.dram_tensor("invcb", [128, groups * G], F32, kind="ExternalInput")
    iota_h = nc.dram_tensor("iota8", [128, 128], F32, kind="ExternalInput")
    wt_h = nc.dram_tensor("wt", [D, D], F32, kind="ExternalInput")
    bb_h = nc.dram_tensor("bb", [1, D], F32, kind="ExternalInput")
    ones_h = nc.dram_tensor("ones", [1, D], F32, kind="ExternalInput")
    out_h = nc.dram_tensor("out", [groups * G, D], F32, kind="ExternalOutput")

    # xg[g] : [128, ST, D] ; device row (p, st) = packed row g*4096+p*32+st,
    # so each partition line is ST*D contiguous elements in HBM.
    xg = xg_h.ap().rearrange("(gr p st) d -> gr p st d", p=128, st=ST)

    mult = mybir.AluOpType.mult
    is_eq = mybir.AluOpType.is_equal

    with tile.TileContext(nc) as tc:
        from contextlib import ExitStack
        with ExitStack() as ctx:
            cpool = ctx.enter_context(tc.tile_pool(name="consts", bufs=1))
            slot_t = cpool.tile([128, groups * ST], F32)
            nc.sync.dma_start(out=slot_t[:], in_=slot_h.ap()[:])
            invcb_t = cpool.tile([128, groups * G], F32)
            nc.sync.dma_start(out=invcb_t[:], in_=invcb_h.ap()[:])
            iota_t = cpool.tile([128, 128], F32)
            nc.sync.dma_start(out=iota_t[:], in_=iota_h.ap()[:])
            wt_t = cpool.tile([D, D], F32)
            nc.sync.dma_start(out=wt_t[:], in_=wt_h.ap()[:])
            bb_t = cpool.tile([1, D], F32)
            nc.sync.dma_start(out=bb_t[:], in_=bb_h.ap()[:])
            ones_t = cpool.tile([1, D], F32)
            nc.sync.dma_start(out=ones_t[:], in_=ones_h.ap()[:])

            pooledT = cpool.tile([128, groups * G], F32)  # persistent result

            xpool = ctx.enter_context(tc.tile_pool(name="x", bufs=3))
            xtpool = ctx.enter_context(tc.tile_pool(name="xt", bufs=3))
            ppool = ctx.enter_context(tc.tile_pool(name="p", bufs=10))
            qpool = ctx.enter_context(tc.tile_pool(name="q", bufs=4))
            ctpool = ctx.enter_context(tc.tile_pool(name="ct", bufs=4))

            with ExitStack() as psctx:
                psa_pool = psctx.enter_context(
                    tc.tile_pool(name="psA", bufs=2, space="PSUM"))
                psq_pool = psctx.enter_context(
                    tc.tile_pool(name="psAq", bufs=3, space="PSUM"))
                psp_pool = psctx.enter_context(
                    tc.tile_pool(name="psPool", bufs=2, space="PSUM"))

                state = {}  # per-group live tiles handed from pass A to B

                def emit_passA(g):
                    xg_t = xpool.tile([128, ST * D], BF16)
                    nc.sync.dma_start(out=xg_t[:], in_=xg[g])
                    xt_t = xtpool.tile([128, GROUP_ROWS], BF16)
                    nc.sync.dma_start(
                        out=xt_t[:],
                        in_=xt_h.ap()[:, g * GROUP_ROWS:(g + 1) * GROUP_ROWS],
                    )
                    # one-hot P for 8 sub-tiles at a time: [128, 8*16]
                    pocts = []
                    for o in range(4):
                        po = ppool.tile([128, 128], BF16, tag="p")
                        nc.vector.tensor_tensor(
                            out=po[:],
                            in0=slot_t[:, g * ST + 8 * o: g * ST + 8 * o + 8]
                                .to_broadcast([128, 8, G]),
                            in1=iota_t[:],
                            op=is_eq,
                        )
                        pocts.append(po)
                    # sumsT[d, slot] += xg_st^T @ P_st   (x stationary)
                    psA = psa_pool.tile([128, G], F32, space="PSUM")
                    for st in range(ST):
                        o, j = st // 8, st % 8
                        nc.tensor.matmul(
                            out=psA[:],
                            lhsT=xg_t[:, st * D:(st + 1) * D],
                            rhs=pocts[o][:, j * G:(j + 1) * G],
                            start=(st == 0),
                            stop=(st == ST - 1),
                        )
                    # ct[d, slot] = sumsT * 1/count  (already transposed)
                    ct = ctpool.tile([128, G], BF16, tag="ct")
                    nc.vector.tensor_tensor(
                        out=ct[:], in0=psA[:],
                        in1=invcb_t[:, g * G:(g + 1) * G], op=mult,
                    )
                    state[g] = (xg_t, xt_t, pocts, ct)

                def emit_passB(g):
                    xg_t, xt_t, pocts, ct = state.pop(g)
                    psPool = psp_pool.tile([128, G], F32, space="PSUM")
                    for o in range(4):
                        # att[rows, slot] for 8 sub-tiles (xt stationary)
                        psAq = psq_pool.tile([128, 128], F32, space="PSUM")
                        for j in range(8):
                            st = 8 * o + j
                            nc.tensor.matmul(
                                out=psAq[:, j * G:(j + 1) * G],
                                lhsT=xt_t[:, st * 128:(st + 1) * 128],
                                rhs=ct[:],
                                start=True, stop=True,
                            )
                        qq = qpool.tile([128, 128], BF16, tag="q")
                        nc.vector.tensor_tensor(
                            out=qq[:], in0=psAq[:], in1=pocts[o][:], op=mult,
                        )
                        # pooledT[d, slot] += xg_st^T @ Q_st  (x stationary)
                        for j in range(8):
                            st = 8 * o + j
                            nc.tensor.matmul(
                                out=psPool[:],
                                lhsT=xg_t[:, st * D:(st + 1) * D],
                                rhs=qq[:, j * G:(j + 1) * G],
                                start=(st == 0),
                                stop=(st == ST - 1),
                            )
                    nc.vector.tensor_tensor(
                        out=pooledT[:, g * G:(g + 1) * G], in0=psPool[:],
                        in1=invcb_t[:, g * G:(g + 1) * G], op=mult,
                    )

                for g in range(groups):
                    emit_passA(g)
                    if g >= 1:
                        emit_passB(g - 1)
                emit_passB(groups - 1)

            # final linear: out[seg, :] = pooled @ W^T + b  (windows of 128)
            with (
                tc.tile_pool(name="psO", bufs=2, space="PSUM") as pso_pool,
                tc.tile_pool(name="ob", bufs=2) as ob_pool,
            ):
                n_segs = groups * G
                for w0 in range(0, n_segs, 128):
                    wn = min(128, n_segs - w0)
                    pso = pso_pool.tile([128, D], F32, space="PSUM")
                    nc.tensor.matmul(
                        out=pso[:wn, :], lhsT=pooledT[:, w0:w0 + wn],
                        rhs=wt_t[:], start=True, stop=False,
                    )
                    nc.tensor.matmul(
                        out=pso[:wn, :], lhsT=ones_t[:, :wn], rhs=bb_t[:],
                        start=False, stop=True,
                    )
                    ob = ob_pool.tile([128, D], F32)
                    nc.vector.tensor_copy(out=ob[:wn, :], in_=pso[:wn, :])
                    nc.sync.dma_start(
                        out=out_h.ap()[w0:w0 + wn, :], in_=ob[:wn, :],
                    )

    return nc


# --------------------------------------------------------------------------
# entry point
# --------------------------------------------------------------------------

def kernel(x, batch, W, b, num_segments):
    global LAST_RESULT
    x = np.asarray(x)
    batch = np.asarray(batch)
    W = np.asarray(W, dtype=np.float32)
    b = np.asarray(b, dtype=np.float32)

    in_maps, seg_ids = _host_prepare(x, batch, W, b)
    nc = _build_program()

    trace = bool(int(os.environ.get("KERNEL_TRACE", "0")))
    res = run_bass_kernel_spmd(
        nc, in_maps, core_ids=list(range(N_CORES)),
        trace=trace, trace_cores=[0] if trace else None,
    )
    LAST_RESULT = res

    out = np.empty((B_SEGS, D), dtype=np.float32)
    packed = np.concatenate([res.results[c]["out"] for c in range(N_CORES)], axis=0)
    out[seg_ids.reshape(-1)] = packed
    return out


# revision 3
# speedup vs baseline: 1.6348x; 1.2286x over previous
"""AttentionPooling (segment mean -> att = <x_i, coarse[batch_i]> -> weighted
segment mean -> Linear) on 8 Trainium2 NeuronCores.

Strategy
--------
`batch` is sorted and host-visible inside kernel(), so ALL index structure is
resolved on the host:

* The 8192 segments are bin-packed into 512 groups of exactly 16 segments,
  each group padded to 4096 rows (32 sub-tiles of 128 rows).  Rows are
  permuted so every group is contiguous; pad rows are zero.  Each core owns
  64 groups -> perfectly uniform SPMD program, no collectives (a segment
  never straddles cores).
* Row (p, st) of a group maps to packed row p*32+st, so the rows-layout DMA
  moves 8 KiB contiguous per partition (128 descriptors per tile instead of
  4096).  The transposed copy is column-permuted on the host to match.
* All matmuls keep x as the 128-column STATIONARY operand (FWL-eligible)
  and stream 16-column moving operands (issue-floor bound, ~25 ns/pair):
    pass A:  sumsT[d, slot]   += xg_st^T @ P_st     (P one-hot, moving)
             ct = sumsT * (1/count)                 [d, 16] bf16, no transpose
    pass B:  att[rows, slot]   = xt_st^T @ ct       (xt stationary)
             Q = att * P        (one DVE multiply per 8 sub-tiles)
             pooledT[d, slot] += xg_st^T @ Q_st
             pooledT slice * (1/count) -> SBUF      (DVE, no transpose)
    final :  out[seg, :] = pooledT^T @ W^T + b      (8 window matmuls)
* x is fed in bf16 in both layouts; all reductions accumulate in fp32 PSUM.
"""

import os

import numpy as np
import ml_dtypes

import concourse.bass as bass
import concourse.mybir as mybir
import concourse.tile as tile
from concourse.bass_utils import run_bass_kernel_spmd
from concourse.vector_clock import ScopedClock

BF16 = mybir.dt.bfloat16
F32 = mybir.dt.float32
F8 = mybir.dt.float8e4
U8 = mybir.dt.uint8

N_CORES = 8
B_SEGS = 8192
D = 128
G = 16                  # segments (slots) per group
ST = 32                 # 128-row sub-tiles per group
GROUP_ROWS = ST * 128   # 4096
N_GROUPS = B_SEGS // G  # 512
GROUPS_PER_CORE = N_GROUPS // N_CORES  # 64
CORE_ROWS = GROUPS_PER_CORE * GROUP_ROWS  # 262144

LAST_RESULT = None  # BassKernelResults of the most recent run (for test.py)

_PATCHED = False


def _patch_tile_tail():
    """The walrus build in this container only lowers ONE sync-wait per
    instruction.  Tile routinely emits multi-wait instructions, so (a) split
    every scheduled instruction's extra waits onto injected same-engine NOPs
    (engines execute their stream in order, so a wait on a preceding NOP is
    equivalent), and (b) do the same for the TileContext exit drain."""
    global _PATCHED
    if _PATCHED:
        return
    _PATCHED = True

    orig_lower = tile.TileContext._lower_ordered_insts

    def _lower_ordered_insts(self, ordered):
        nid = [0]
        for bb_name, insts in ordered.items():
            new = []
            for inst in insts:
                si = inst.sync_info
                if si is not None and si.on_wait and len(si.on_wait) > 1:
                    waits = list(si.on_wait)
                    for w in waits[:-1]:
                        nid[0] += 1
                        nop = mybir.InstNoOp(
                            name=f"splitw-{nid[0]}-{inst.name}",
                            engine=inst.engine,
                            sync_info=mybir.SyncInfo(on_wait=[w], on_update=[]),
                            bass_nofuse=True,
                        )
                        new.append(nop)
                    si.on_wait = [waits[-1]]
                new.append(inst)
            ordered[bb_name] = new
        return orig_lower(self, ordered)

    tile.TileContext._lower_ordered_insts = _lower_ordered_insts

    def _drain_and_barrier(self, tick_clock, wait_clock):
        nc = self.nc
        probe = nc.sync.nop(nofuse=True, hint="tail_wait0")
        wait_clock.add_sem_waits(
            probe.ins, ScopedClock({None: tick_clock.global_clock})
        )
        si = probe.ins.sync_info
        waits = list(si.on_wait or []) if si is not None else []
        if len(waits) > 1:
            si.on_wait = waits[:1]
            for w in waits[1:]:
                n2 = nc.sync.nop(nofuse=True, hint="tail_wait")
                n2.ins.sync_info = mybir.SyncInfo(on_wait=[w], on_update=[])
        nc.sync.drain()
        nc.all_engine_barrier()
        popped = nc._tile_sem_poison_stack.pop()
        assert popped is self._sem_poison
        nc.clear_and_free_semaphores(list(self.sems.allocated().values()))
        nc.all_engine_barrier()

    tile.TileContext._drain_and_barrier = _drain_and_barrier


# --------------------------------------------------------------------------
# host-side packing
# --------------------------------------------------------------------------

def _pack_segments(counts):
    """Assign each segment to a (group, slot).  512 groups x 16 slots, rows
    per group <= GROUP_ROWS.  Balanced LPT dealing: 16 rounds; each round
    hands the next 512 largest segments to the currently lightest groups."""
    order = np.argsort(-counts, kind="stable")
    loads = np.zeros(N_GROUPS, dtype=np.int64)
    seg_ids = np.empty((N_GROUPS, G), dtype=np.int64)
    for r in range(G):
        chunk = order[r * N_GROUPS:(r + 1) * N_GROUPS]
        grp_order = np.argsort(loads, kind="stable")
        seg_ids[grp_order, r] = chunk
        loads[grp_order] += counts[chunk]
    assert loads.max() <= GROUP_ROWS, (
        f"group overflow: {loads.max()} > {GROUP_ROWS}"
    )
    return seg_ids  # [512, 16] segment id per (group, slot)


def _host_prepare(x, batch, W, b):
    counts = np.bincount(batch, minlength=B_SEGS).astype(np.int64)
    seg_start = np.concatenate([[0], np.cumsum(counts)[:-1]])
    seg_ids = _pack_segments(counts)                       # [512, 16]

    flat_segs = seg_ids.reshape(-1)                        # packed order
    flat_counts = counts[flat_segs]
    # destination start of each packed segment
    within = flat_counts.reshape(N_GROUPS, G)
    offs = np.cumsum(within, axis=1) - within              # [512, 16]
    dest_start = (np.arange(N_GROUPS)[:, None] * GROUP_ROWS + offs).reshape(-1)
    src_start = seg_start[flat_segs]

    total = int(flat_counts.sum())
    assert total == x.shape[0]
    rag = np.arange(total, dtype=np.int64) - np.repeat(
        np.cumsum(flat_counts) - flat_counts, flat_counts
    )
    valid_dest = np.repeat(dest_start, flat_counts) + rag
    valid_src = np.repeat(src_start, flat_counts) + rag

    n_pad = N_GROUPS * GROUP_ROWS
    x_bf = x.astype(ml_dtypes.bfloat16)
    x_pad = np.zeros((n_pad, D), dtype=ml_dtypes.bfloat16)
    x_pad[valid_dest] = x_bf[valid_src]
    # Device row (p, st) of group g = packed row g*4096 + p*32 + st:
    # the rows layout is x_pad itself (8 KiB contiguous per partition); the
    # transposed copy is column-permuted to st-major so sub-tile st's
    # columns are the 128 rows {p*32+st}.
    xt_pad = np.ascontiguousarray(
        x_pad.reshape(N_GROUPS, 128, ST, D)
        .transpose(0, 2, 1, 3)
        .reshape(n_pad, D)
        .T
    ).astype(ml_dtypes.float8_e4m3)                        # [128, n_pad]

    slotvec = np.zeros(n_pad, dtype=np.uint8)
    slot_of_seg = np.repeat(
        np.tile(np.arange(G, dtype=np.uint8), N_GROUPS), flat_counts
    )
    slotvec[valid_dest] = slot_of_seg
    # slot[p, g*ST+st] = slotvec[g*4096 + p*32 + st]
    slot_pmaj = (
        slotvec.reshape(N_GROUPS, 128, ST)
        .transpose(1, 0, 2)
        .reshape(128, N_GROUPS * ST)
    )

    invc = (1.0 / np.maximum(counts, 1)).astype(np.float32)
    invc_packed = invc[flat_segs].reshape(N_GROUPS, G)     # [512, 16]

    # iota8[p, 16*j + s] = s  (compare target for 8 sub-tiles' one-hots)
    iota8 = np.tile(
        np.tile(np.arange(G, dtype=np.uint8), 8)[None, :], (128, 1)
    )                                                      # [128, 128]
    consts = {
        "iota8": np.ascontiguousarray(iota8),
        "wt": np.ascontiguousarray(W.T.astype(np.float32)),
        "bb": np.ascontiguousarray(b.astype(np.float32).reshape(1, D)),
        "ones": np.ones((1, D), dtype=np.float32),
    }

    in_maps = []
    for c in range(N_CORES):
        r0, r1 = c * CORE_ROWS, (c + 1) * CORE_ROWS
        g0, g1 = c * GROUPS_PER_CORE, (c + 1) * GROUPS_PER_CORE
        m = {
            "xg": np.ascontiguousarray(x_pad[r0:r1]),
            "xt": np.ascontiguousarray(xt_pad[:, r0:r1]),
            "slot": np.ascontiguousarray(
                slot_pmaj[:, g0 * ST:g1 * ST]
            ),
            # invcb[p, g_local*16 + s] = 1/count of (group, slot)
            "invcb": np.ascontiguousarray(
                np.tile(
                    invc_packed[g0:g1].reshape(1, -1), (128, 1)
                )
            ),
        }
        m.update(consts)
        in_maps.append(m)

    return in_maps, seg_ids


# --------------------------------------------------------------------------
# device program
# --------------------------------------------------------------------------

def _build_program(groups=GROUPS_PER_CORE):
    _patch_tile_tail()
    nc = bass.Bass("TRN2", debug=False)
    rows = groups * GROUP_ROWS

    xg_h = nc.dram_tensor("xg", [rows, D], BF16, kind="ExternalInput")
    xt_h = nc.dram_tensor("xt", [D, rows], F8, kind="ExternalInput")
    slot_h = nc.dram_tensor("slot", [128, groups * ST], U8, kind="ExternalInput")
    invcb_h = nc.dram_tensor("invcb", [128, groups * G], F32, kind="ExternalInput")
    iota_h = nc.dram_tensor("iota8", [128, 128], U8, kind="ExternalInput")
    wt_h = nc.dram_tensor("wt", [D, D], F32, kind="ExternalInput")
    bb_h = nc.dram_tensor("bb", [1, D], F32, kind="ExternalInput")
    ones_h = nc.dram_tensor("ones", [1, D], F32, kind="ExternalInput")
    out_h = nc.dram_tensor("out", [groups * G, D], F32, kind="ExternalOutput")

    # xg[g] : [128, ST, D] ; device row (p, st) = packed row g*4096+p*32+st,
    # so each partition line is ST*D contiguous elements in HBM.
    xg = xg_h.ap().rearrange("(gr p st) d -> gr p st d", p=128, st=ST)

    mult = mybir.AluOpType.mult
    is_eq = mybir.AluOpType.is_equal

    with tile.TileContext(nc) as tc:
        from contextlib import ExitStack
        with ExitStack() as ctx:
            cpool = ctx.enter_context(tc.tile_pool(name="consts", bufs=1))
            slot_t = cpool.tile([128, groups * ST], U8)
            nc.sync.dma_start(out=slot_t[:], in_=slot_h.ap()[:])
            invcb_t = cpool.tile([128, groups * G], F32)
            nc.sync.dma_start(out=invcb_t[:], in_=invcb_h.ap()[:])
            iota_t = cpool.tile([128, 128], U8)
            nc.sync.dma_start(out=iota_t[:], in_=iota_h.ap()[:])
            wt_t = cpool.tile([D, D], F32)
            nc.sync.dma_start(out=wt_t[:], in_=wt_h.ap()[:])
            bb_t = cpool.tile([1, D], F32)
            nc.sync.dma_start(out=bb_t[:], in_=bb_h.ap()[:])
            ones_t = cpool.tile([1, D], F32)
            nc.sync.dma_start(out=ones_t[:], in_=ones_h.ap()[:])

            pooledT = cpool.tile([128, groups * G], F32)  # persistent result

            xpool = ctx.enter_context(tc.tile_pool(name="x", bufs=3))
            xtpool = ctx.enter_context(tc.tile_pool(name="xt", bufs=3))
            ppool = ctx.enter_context(tc.tile_pool(name="p", bufs=10))
            qpool = ctx.enter_context(tc.tile_pool(name="q", bufs=4))
            ctpool = ctx.enter_context(tc.tile_pool(name="ct", bufs=4))

            with ExitStack() as psctx:
                psa_pool = psctx.enter_context(
                    tc.tile_pool(name="psA", bufs=2, space="PSUM"))
                psq_pool = psctx.enter_context(
                    tc.tile_pool(name="psAq", bufs=3, space="PSUM"))
                psp_pool = psctx.enter_context(
                    tc.tile_pool(name="psPool", bufs=2, space="PSUM"))

                state = {}  # per-group live tiles handed from pass A to B

                def emit_passA(g):
                    xg_t = xpool.tile([128, ST * D], BF16)
                    nc.sync.dma_start(out=xg_t[:], in_=xg[g])
                    xt_t = xtpool.tile([128, GROUP_ROWS], F8)
                    nc.sync.dma_start(
                        out=xt_t[:],
                        in_=xt_h.ap()[:, g * GROUP_ROWS:(g + 1) * GROUP_ROWS],
                    )
                    # one-hot P for 8 sub-tiles at a time: [128, 8*16]
                    pocts = []
                    for o in range(4):
                        po = ppool.tile([128, 128], BF16, tag="p")
                        nc.vector.tensor_tensor(
                            out=po[:],
                            in0=slot_t[:, g * ST + 8 * o: g * ST + 8 * o + 8]
                                .to_broadcast([128, 8, G]),
                            in1=iota_t[:],
                            op=is_eq,
                        )
                        pocts.append(po)
                    # sumsT[d, slot] += xg_st^T @ P_st   (x stationary)
                    psA = psa_pool.tile([128, G], F32, space="PSUM")
                    for st in range(ST):
                        o, j = st // 8, st % 8
                        nc.tensor.matmul(
                            out=psA[:],
                            lhsT=xg_t[:, st * D:(st + 1) * D],
                            rhs=pocts[o][:, j * G:(j + 1) * G],
                            start=(st == 0),
                            stop=(st == ST - 1),
                        )
                    # ct[d, slot] = sumsT * 1/count  (already transposed)
                    ct = ctpool.tile([128, G], BF16, tag="ct")
                    nc.vector.tensor_tensor(
                        out=ct[:], in0=psA[:],
                        in1=invcb_t[:, g * G:(g + 1) * G], op=mult,
                    )
                    state[g] = (xg_t, xt_t, pocts, ct)

                def emit_passB(g):
                    xg_t, xt_t, pocts, ct = state.pop(g)
                    psPool = psp_pool.tile([128, G], F32, space="PSUM")
                    for o in range(4):
                        # att[rows, slot] for 8 sub-tiles (xt stationary)
                        psAq = psq_pool.tile([128, 128], F32, space="PSUM")
                        for j in range(8):
                            st = 8 * o + j
                            nc.tensor.matmul(
                                out=psAq[:, j * G:(j + 1) * G],
                                lhsT=xt_t[:, st * 128:(st + 1) * 128],
                                rhs=ct[:],
                                start=True, stop=True,
                            )
                        qq = qpool.tile([128, 128], BF16, tag="q")
                        nc.vector.tensor_tensor(
                            out=qq[:], in0=psAq[:], in1=pocts[o][:], op=mult,
                        )
                        # pooledT[d, slot] += xg_st^T @ Q_st  (x stationary)
                        for j in range(8):
                            st = 8 * o + j
                            nc.tensor.matmul(
                                out=psPool[:],
                                lhsT=xg_t[:, st * D:(st + 1) * D],
                                rhs=qq[:, j * G:(j + 1) * G],
                                start=(st == 0),
                                stop=(st == ST - 1),
                            )
                    nc.vector.tensor_tensor(
                        out=pooledT[:, g * G:(g + 1) * G], in0=psPool[:],
                        in1=invcb_t[:, g * G:(g + 1) * G], op=mult,
                    )

                for g in range(groups):
                    emit_passA(g)
                    if g >= 1:
                        emit_passB(g - 1)
                emit_passB(groups - 1)

            # final linear: out[seg, :] = pooled @ W^T + b  (windows of 128)
            with (
                tc.tile_pool(name="psO", bufs=2, space="PSUM") as pso_pool,
                tc.tile_pool(name="ob", bufs=2) as ob_pool,
            ):
                n_segs = groups * G
                for w0 in range(0, n_segs, 128):
                    wn = min(128, n_segs - w0)
                    pso = pso_pool.tile([128, D], F32, space="PSUM")
                    nc.tensor.matmul(
                        out=pso[:wn, :], lhsT=pooledT[:, w0:w0 + wn],
                        rhs=wt_t[:], start=True, stop=False,
                    )
                    nc.tensor.matmul(
                        out=pso[:wn, :], lhsT=ones_t[:, :wn], rhs=bb_t[:],
                        start=False, stop=True,
                    )
                    ob = ob_pool.tile([128, D], F32)
                    nc.vector.tensor_copy(out=ob[:wn, :], in_=pso[:wn, :])
                    nc.sync.dma_start(
                        out=out_h.ap()[w0:w0 + wn, :], in_=ob[:wn, :],
                    )

    return nc


# --------------------------------------------------------------------------
# entry point
# --------------------------------------------------------------------------

def kernel(x, batch, W, b, num_segments):
    global LAST_RESULT
    x = np.asarray(x)
    batch = np.asarray(batch)
    W = np.asarray(W, dtype=np.float32)
    b = np.asarray(b, dtype=np.float32)

    in_maps, seg_ids = _host_prepare(x, batch, W, b)
    nc = _build_program()

    trace = bool(int(os.environ.get("KERNEL_TRACE", "0")))
    res = run_bass_kernel_spmd(
        nc, in_maps, core_ids=list(range(N_CORES)),
        trace=trace, trace_cores=[0] if trace else None,
    )
    LAST_RESULT = res

    out = np.empty((B_SEGS, D), dtype=np.float32)
    packed = np.concatenate([res.results[c]["out"] for c in range(N_CORES)], axis=0)
    out[seg_ids.reshape(-1)] = packed
    return out
